# revision 15
# baseline (speedup 1.0000x reference)
"""Multi-head attention (B=2, S=2048, D=1024, H=16) on 8 NeuronCores.

Sharding: core c -> batch b = c//4, head group g = c%4 (4 heads each).
Each core computes q/k/v projections for its 4 heads, attention, and a
partial out-projection (2048, 1024). Host sums the 4 partials per batch
and adds the output bias.

Per-core layouts (all fp32 bits; matmul operands declared float32r so the
PE runs at 1 cycle/row instead of 4):
  xt  (1024, 2048)  = x[b].T                 ([k, s], k on partitions)
  wq/wk/wv (1024, 256) = W[gs:gs+256, :].T   ([k, n])
  wo  (256, 1024)   = Wo[:, gs:gs+256].T     ([n, j])
  bq/bk/bv (256,)
qT/kT live on-chip as [n, s] (head-dim on partitions) so scores come out
transposed [t, s]; softmax needs no max-subtraction (|scores/8| < ~2).
The softmax denominator falls out of the weighted matmul by appending a
ones-column to v (lhsT = [v_h | 1], M=65): psum row 64 = sum_t probs.
Normalization multiplies by the gpsimd-partition-broadcast reciprocal.

Phase structure (the PE instruction stream is in emission order, so
emission order is the schedule): projections for heads {0,1} + v first;
attention for heads {0,1} with the heads-{2,3} projection chunks
interleaved into its ACT-bound exp stream; attention for heads {2,3}
with the out-projection s-tiles interleaved the same way.
"""

import sys

if "/opt/trn_rl_repo" not in sys.path:
    sys.path.insert(0, "/opt/trn_rl_repo")

from contextlib import ExitStack

import numpy as np

import concourse.bacc as bacc
import concourse.tile as tile
from concourse import mybir
from concourse.bass_utils import run_bass_kernel_spmd

F32 = mybir.dt.float32
R32 = mybir.dt.float32r  # reduced-precision fp32 PE mode, 1 cyc/row
Exp = mybir.ActivationFunctionType.Exp
Identity = mybir.ActivationFunctionType.Identity

S = 2048
D = 1024
NH = 256  # head cols per core (4 heads x 64)
KT = D // 128  # 8 contraction tiles
ST = S // 128  # 16 seq tiles
SC = S // 512  # 4 s-chunks of 512

_CACHE = {}


def _build():
    if "nc" in _CACHE:
        return _CACHE["nc"]

    nc = bacc.Bacc(None, target_bir_lowering=False)

    xt_h = nc.dram_tensor("xt", [D, S], R32, kind="ExternalInput")
    wq_h = nc.dram_tensor("wq", [D, NH], R32, kind="ExternalInput")
    wk_h = nc.dram_tensor("wk", [D, NH], R32, kind="ExternalInput")
    wv_h = nc.dram_tensor("wv", [D, NH], R32, kind="ExternalInput")
    wo_h = nc.dram_tensor("wo", [NH, D], R32, kind="ExternalInput")
    bq_h = nc.dram_tensor("bq", [NH], F32, kind="ExternalInput")
    bk_h = nc.dram_tensor("bk", [NH], F32, kind="ExternalInput")
    bv_h = nc.dram_tensor("bv", [NH], F32, kind="ExternalInput")
    out_h = nc.dram_tensor("out", [S, D], F32, kind="ExternalOutput")

    with ExitStack() as top:
        tc = top.enter_context(tile.TileContext(nc))

        qkv = top.enter_context(tc.tile_pool(name="qkv", bufs=1))
        npool = top.enter_context(tc.tile_pool(name="npool", bufs=1))
        wop = top.enter_context(tc.tile_pool(name="wop", bufs=1))
        probs_p = top.enter_context(tc.tile_pool(name="probs", bufs=4))
        den_p = top.enter_context(tc.tile_pool(name="den", bufs=2))
        psc = top.enter_context(tc.tile_pool(name="psc", bufs=2, space="PSUM"))
        pw = top.enter_context(tc.tile_pool(name="pw", bufs=1, space="PSUM"))

        q01 = qkv.tile([128, S], R32, tag="q01")
        q23 = qkv.tile([128, S], R32, tag="q23")
        k01 = qkv.tile([128, S], R32, tag="k01")
        k23 = qkv.tile([128, S], R32, tag="k23")
        v5 = qkv.tile([128, ST, 4 * 65], R32, tag="v5")
        n01 = npool.tile([128, S], R32, tag="n01")
        n23 = npool.tile([128, S], R32, tag="n23")
        wo01 = wop.tile([128, D], R32, tag="wo01")
        wo23 = wop.tile([128, D], R32, tag="wo23")

        def attention(pair, qp, kp, ntp, hooks):
            """hooks: {iter_index: [closure, ...]} emitted into the stream."""
            for q in range(SC):
                wt = [
                    pw.tile([65, 512], F32, name=f"wt{hi}", tag=f"wt{hi}")
                    for hi in range(2)
                ]
                # one-step software pipeline: weighted(t-1) is emitted
                # AFTER scores(t)+exp(t), so the in-order PE stream never
                # blocks the next scores behind a weighted that waits on
                # the current exp -- ACT runs gap-free
                def weighted(t, pr):
                    for hi in range(2):
                        h = 2 * pair + hi
                        nc.tensor.matmul(
                            wt[hi],
                            v5[:, t, 65 * h : 65 * h + 65],
                            pr[:, 512 * hi : 512 * hi + 512],
                            start=(t == 0),
                            stop=(t == ST - 1),
                        )

                prev = None
                for t in range(ST):
                    for fn in hooks.get(q * ST + t, ()):
                        fn()
                    sc_ps = psc.tile([128, 1024], F32, name="scores", tag="scores")
                    for hi in range(2):
                        nc.tensor.matmul(
                            sc_ps[:, 512 * hi : 512 * hi + 512],
                            kp[64 * hi : 64 * hi + 64, 128 * t : 128 * t + 128],
                            qp[64 * hi : 64 * hi + 64, 512 * q : 512 * q + 512],
                            start=True,
                            stop=True,
                            tile_position=(64 * hi, 0),
                        )
                    pr = probs_p.tile([128, 1024], R32, name="probs", tag="probs")
                    nc.scalar.activation(pr, sc_ps, Exp, scale=0.125)
                    if prev is not None:
                        weighted(t - 1, prev)
                    prev = pr
                weighted(ST - 1, prev)
                # stage-parallel normalize: both recips back-to-back on
                # DVE, both broadcasts on gpsimd (overlapping DVE), both
                # muls -- frees the wt accumulators for the next q-chunk
                # ~0.6us sooner than per-head serial chains
                dens, bcs = [], []
                for hi in range(2):
                    den_r = den_p.tile([1, 512], F32, name="den", tag="den")
                    nc.vector.reciprocal(den_r, wt[hi][64:65, :])
                    dens.append(den_r)
                for hi in range(2):
                    recipB = den_p.tile([64, 512], F32, name="recipB", tag="recipB")
                    nc.gpsimd.partition_broadcast(recipB, dens[hi])
                    bcs.append(recipB)
                for hi in range(2):
                    nc.vector.tensor_mul(
                        ntp[64 * hi : 64 * hi + 64, 512 * q : 512 * q + 512],
                        wt[hi][0:64, :],
                        bcs[hi],
                    )

        with (
            tc.tile_pool(name="wpool", bufs=1) as wpool,
            tc.tile_pool(name="xpool", bufs=1) as xpool,
            tc.tile_pool(name="pproj", bufs=2, space="PSUM") as pproj,
        ):
            wq_sb = wpool.tile([128, KT, NH], R32, tag="wq")
            wk_sb = wpool.tile([128, KT, NH], R32, tag="wk")
            wv_sb = wpool.tile([128, KT, NH], R32, tag="wv")
            bq_sb = wpool.tile([128, 2], F32, tag="bq")
            bk_sb = wpool.tile([128, 2], F32, tag="bk")
            bv1 = wpool.tile([1, NH], F32, tag="bv1")
            bvB = wpool.tile([128, NH], F32, tag="bvB")
            xt_sb = xpool.tile([128, KT, S], R32, tag="xt")

            # DMA order = transfer priority: weights for k/q, biases,
            # then xt per-(k, s-chunk) so each projection chunk's operand
            # set arrives as a unit and chunks complete staggered during
            # the DMA instead of holding PSUM accumulators until the end
            def xt_chunk_dma(sc):
                for k in range(KT):
                    nc.sync.dma_start(
                        out=xt_sb[:, k, 512 * sc : 512 * (sc + 1)],
                        in_=xt_h[128 * k : 128 * (k + 1), 512 * sc : 512 * (sc + 1)],
                    )

            for w_sb, w_hbm in ((wk_sb, wk_h), (wq_sb, wq_h)):
                nc.sync.dma_start(
                    out=w_sb, in_=w_hbm.rearrange("(t p) n -> p t n", p=128)
                )
            nc.sync.dma_start(out=bq_sb, in_=bq_h.rearrange("(a p) -> p a", p=128))
            nc.sync.dma_start(out=bk_sb, in_=bk_h.rearrange("(a p) -> p a", p=128))
            nc.sync.dma_start(out=bv1, in_=bv_h.rearrange("(a n) -> a n", a=1))
            nc.gpsimd.partition_broadcast(bvB, bv1)
            xt_chunk_dma(0)
            nc.sync.dma_start(
                out=wv_sb, in_=wv_h.rearrange("(t p) n -> p t n", p=128)
            )
            for sc in range(1, SC):
                xt_chunk_dma(sc)
            nc.sync.dma_start(out=wo01, in_=wo_h[0:128, :])
            nc.sync.dma_start(out=wo23, in_=wo_h[128:256, :])

            # ones columns of v5 (memset can't write f32r; ACT conversion can)
            v5ones = v5.rearrange("p t (h c) -> p (t h) c", c=65)[:, :, 64:65]
            ones_src = bvB[:, 0:64].rearrange("p (a b) -> p a b", b=1)
            nc.scalar.activation(v5ones, ones_src, Identity, bias=1.0, scale=0.0)

            def qk_half(pt, w_sb, b_sb, nh, dst, sc, half):
                for k in range(4 * half, 4 * half + 4):
                    nc.tensor.matmul(
                        pt,
                        w_sb[:, k, 128 * nh : 128 * (nh + 1)],
                        xt_sb[:, k, 512 * sc : 512 * (sc + 1)],
                        start=(k == 0),
                        stop=(k == KT - 1),
                    )
                if half == 1:
                    nc.vector.tensor_scalar_add(
                        dst[:, 512 * sc : 512 * (sc + 1)], pt, b_sb[:, nh : nh + 1]
                    )

            def qk_chunk(w_sb, b_sb, nh, dst, sc):
                pt = pproj.tile([128, 512], F32, name="pproj", tag="pproj")
                qk_half(pt, w_sb, b_sb, nh, dst, sc, 0)
                qk_half(pt, w_sb, b_sb, nh, dst, sc, 1)

            def qk_chunk_split(w_sb, b_sb, nh, dst, sc):
                # two consecutive hook slots share one psum accumulator so
                # each slot inserts only ~4 matmuls into the exp stream
                pt = pproj.tile([128, 512], F32, name="pproj", tag="pproj")
                return (
                    lambda: qk_half(pt, w_sb, b_sb, nh, dst, sc, 0),
                    lambda: qk_half(pt, w_sb, b_sb, nh, dst, sc, 1),
                )

            def v_chunk(t):
                pt = pproj.tile([128, 512], F32, name="pproj", tag="pproj")
                pvt = pt[:, 0:NH]
                for k in range(KT):
                    nc.tensor.matmul(
                        pvt,
                        xt_sb[:, k, 128 * t : 128 * (t + 1)],
                        wv_sb[:, k, :],
                        start=(k == 0),
                        stop=(k == KT - 1),
                    )
                v5t = v5[:, t, :].rearrange("p (h c) -> p h c", c=65)
                nc.vector.tensor_add(
                    v5t[:, :, 0:64],
                    pvt.rearrange("p (h c) -> p h c", c=64),
                    bvB.rearrange("p (h c) -> p h c", c=64),
                )

            # phase A1: just enough for attention(pair0, q=0, t<4);
            # everything else streams in through pair-0 hooks, just in time
            qk_chunk(wk_sb, bk_sb, 0, k01, 0)
            qk_chunk(wq_sb, bq_sb, 0, q01, 0)
            for t in range(4):
                v_chunk(t)

            hooks = {}

            def at(it, fn):
                hooks.setdefault(it, []).append(fn)

            def at_split(it, parts):
                at(it, parts[0])
                at(it + 1, parts[1])

            for sc in range(1, SC):  # k01 sc used from t = 4*sc on
                at_split(4 * sc - 3, qk_chunk_split(wk_sb, bk_sb, 0, k01, sc))
            for t in range(4, ST):  # v5[t] used by weighted(q0, t)
                at(t, lambda t=t: v_chunk(t))
            for q in range(1, SC):  # q01 sc=q used from iter 16*q on
                at_split(16 * q - 4, qk_chunk_split(wq_sb, bq_sb, 0, q01, q))
            # heads-{2,3} projections anywhere in q1..q3
            for i in range(SC):
                at_split(16 + 4 * i + 1, qk_chunk_split(wk_sb, bk_sb, 1, k23, i))
                at_split(32 + 4 * i + 1, qk_chunk_split(wq_sb, bq_sb, 1, q23, i))
            attention(0, q01, k01, n01, hooks)

        # pair-1 attention with out-projection s-tiles interleaved
        with (
            tc.tile_pool(name="po", bufs=1, space="PSUM") as po,
            tc.tile_pool(name="osb", bufs=3) as osb,
        ):

            def out_tile(st, pool=None, tag="po"):
                pot = (pool or po).tile([128, D], F32, name="po", tag=tag)
                for j in range(2):
                    nc.tensor.matmul(
                        pot[:, 512 * j : 512 * (j + 1)],
                        n01[:, 128 * st : 128 * (st + 1)],
                        wo01[:, 512 * j : 512 * (j + 1)],
                        start=True,
                        stop=False,
                    )
                for j in range(2):
                    nc.tensor.matmul(
                        pot[:, 512 * j : 512 * (j + 1)],
                        n23[:, 128 * st : 128 * (st + 1)],
                        wo23[:, 512 * j : 512 * (j + 1)],
                        start=False,
                        stop=True,
                    )
                ot = osb.tile([128, D], F32, name="osb", tag="osb")
                nc.vector.tensor_copy(ot, pot)
                nc.sync.dma_start(out=out_h[128 * st : 128 * (st + 1), :], in_=ot)

            # out-proj of q-chunk q becomes available after pair-1 q's
            # normalize; schedule its 4 s-tiles into the next q's t-loop
            hooks = {}
            for q in range(SC - 1):
                for i in range(4):
                    st = 4 * q + i
                    hooks.setdefault((q + 1) * ST + 4 * i + 2, []).append(
                        lambda st=st: out_tile(st)
                    )
            attention(1, q23, k23, n23, hooks)
            for st in range(4 * (SC - 1), 4 * SC):
                out_tile(st, pool=psc, tag="scores")

    nc.compile()
    _CACHE["nc"] = nc
    return nc


def make_in_maps(x, Wq, bq, Wk, bk, Wv, bv, Wo):
    in_maps = []
    for c in range(8):
        b, g = c // 4, c % 4
        cs = NH * g
        in_maps.append(
            {
                "xt": np.ascontiguousarray(x[b].T),
                "wq": np.ascontiguousarray(Wq[cs : cs + NH, :].T),
                "wk": np.ascontiguousarray(Wk[cs : cs + NH, :].T),
                "wv": np.ascontiguousarray(Wv[cs : cs + NH, :].T),
                "wo": np.ascontiguousarray(Wo[:, cs : cs + NH].T),
                "bq": np.ascontiguousarray(bq[cs : cs + NH]),
                "bk": np.ascontiguousarray(bk[cs : cs + NH]),
                "bv": np.ascontiguousarray(bv[cs : cs + NH]),
            }
        )
    return in_maps


def kernel(x, Wq, bq, Wk, bk, Wv, bv, Wo, bo):
    x = np.asarray(x, np.float32)
    args = [np.asarray(a, np.float32) for a in (Wq, bq, Wk, bk, Wv, bv, Wo)]
    bo = np.asarray(bo, np.float32)
    nc = _build()
    in_maps = make_in_maps(x, *args)
    res = run_bass_kernel_spmd(nc, in_maps, core_ids=list(range(8)))
    outs = [res.results[c]["out"] for c in range(8)]
    out = np.stack(
        [
            outs[0] + outs[1] + outs[2] + outs[3] + bo,
            outs[4] + outs[5] + outs[6] + outs[7] + bo,
        ]
    )
    return out.astype(np.float32)
